# revision 6
# baseline (speedup 1.0000x reference)
"""Trainium2 Bass kernel for batched multi-head attention (no scale).

Problem: q,k,v [B=4, H=16, S=2048, D=128] fp32;
    out = softmax(q @ k^T) @ v   (no 1/sqrt(D) scaling)

Sharding: B*H = 64 heads, 8 heads per core across 8 NeuronCores.

v3 design (vs. the P-export baseline at 279.8us):
  The baseline was a three-way tie: ScalarE exp 257.7us, PE matmul 258.1us,
  DMA 88.3MB ~236us (67MB of it the full-P export for the host-side softmax
  denominator).  v3 removes the P export and rebalances:

  * Denominator on device: P strips are pairwise tree-folded on the (idle)
    Vector engine in bf16 (validated: adds <1e-4 to rel err) and the folded
    [128, 1024] tile per supertile is exported (4MB/core); the host does the
    final 128-partition fold + divide.  DMA drops to ~25MB/core.
  * ScalarE (the unavoidable bottleneck: exp is 1 elem/cycle/lane,
    ScalarE-only, ~218us floor + ~154ns/instr):  exp ACTs cover 1.5
    kk-blocks (FD=1536) by reading 3 one-bank half-slots of a persistent
    [128, 6, 512] fp32 PSUM ring with one AP.  Chunks advance by 3 of 6
    positions, so QK always runs a full chunk ahead -- no ACT stalls.
    Supertiles are 32 half-slots (not /6); alternating supertiles shift the
    position map by +3 so chunk position sets stay contiguous with no
    pipeline bubble at boundaries.  ScalarE ~258 -> ~245us.
  * PE: q-supertiles of 1024 make every stationary operand (K-block for QK,
    V-block for AV) serve 2 N=512 matmuls, halving LDWEIGHTS pressure.
  * Warmup: dummy matmuls + a dummy exp during the initial DMA wait warm the
    PE HAM clock-gate (1.2->2.4GHz) and preload the ACT exp table.

dtype choices: Q,K bf16, V fp16, P bf16 (rel err ~8.3e-3, gate 2e-2).
Host pre-transposes Q,K to [D,S] and pre-swizzles V to [128, NKB, D] fp16;
post-applies out = (out^T / l)^T with l from the exported folded P.
"""

import os

import ml_dtypes
import numpy as np

import concourse.bass as bass
import concourse.tile as tile
from concourse import bacc, mybir
from concourse.bass_utils import run_bass_kernel_spmd

B, H, S, D = 4, 16, 2048, 128
N_CORES = 8
HPC = (B * H) // N_CORES  # heads per core
QT = 1024                 # q-supertile width
NQT = S // QT             # 2 supertiles per head
KB = 128                  # kk block (contraction of one matmul)
NKB = S // KB             # 16 kk blocks
NHS = 2 * NKB             # 32 half-slots per supertile
EXP_BIAS = -64.0
F32 = mybir.dt.float32
BF16 = mybir.dt.bfloat16
FP16 = mybir.dt.float16

_NC_CACHE = None


def _build_nc():
    nc = bacc.Bacc("TRN2", target_bir_lowering=False, debug=False)

    qT_d = nc.dram_tensor("qT", [HPC, D, S], BF16, kind="ExternalInput")
    kT_d = nc.dram_tensor("kT", [HPC, D, S], BF16, kind="ExternalInput")
    v_d = nc.dram_tensor("v", [HPC, 128, NKB, D], FP16, kind="ExternalInput")
    oT_d = nc.dram_tensor("outT", [HPC, D, S], F32, kind="ExternalOutput")
    accf_d = nc.dram_tensor(
        "acc_fold", [HPC, NQT, 128, QT], BF16, kind="ExternalOutput"
    )

    # global chunk list: per supertile, 10 chunks of 3 half-slots + 1 of 2.
    # position map pos(h) = (h + 3*(stg%2)) % 6 keeps every chunk's position
    # set contiguous (start 0 or 3) across supertile boundaries.
    chunks = []
    for stg in range(HPC * NQT):
        hd, sti = divmod(stg, NQT)
        shift = 3 * (stg % 2)
        for cs in range(0, NHS - 2, 3):
            chunks.append((hd, sti, stg, shift, cs, 3))
        chunks.append((hd, sti, stg, shift, NHS - 2, 2))
    NCH = len(chunks)
    CPS = NHS // 3 + 1  # 11 chunks per supertile

    with tile.TileContext(nc) as tc:
        with (
            tc.tile_pool(name="io", bufs=2) as io,
            tc.tile_pool(name="pexp", bufs=4) as pexp,
            tc.tile_pool(name="fold", bufs=2) as foldp,
            tc.tile_pool(name="osb", bufs=2) as osbp,
            tc.tile_pool(name="small", bufs=1) as small,
            tc.tile_pool(name="ps", bufs=1, space="PSUM") as ps,
        ):
            bias_sb = small.tile([128, 1], F32)
            nc.vector.memset(bias_sb[:], EXP_BIAS)

            # --- warmup: PE HAM + ACT exp table, during the initial DMA wait
            wu_w = small.tile([128, 128], BF16, name="wu_w")
            wu_r = small.tile([128, 512], BF16, name="wu_r")
            wu_o = small.tile([128, 128], BF16, name="wu_o")
            nc.vector.memset(wu_w[:], 0.0)
            nc.vector.memset(wu_r[:], 0.0)

            # persistent PSUM: 6 half-slots (6 banks) + AV accumulator (2)
            st6 = ps.tile([128, 6, 512], F32, tag="st6", bufs=1, name="st6")

            # ACT table preload (no data deps)
            nc.scalar.activation(
                wu_o[:],
                wu_w[:],
                mybir.ActivationFunctionType.Exp,
                bias=bias_sb[:, :],
                scale=1.0,
            )
            # PE warmup matmuls (~4us of PE activity from t=0)
            for i in range(18):
                nc.tensor.matmul(
                    st6[:, i % 6, :], wu_w[:], wu_r[:], start=True, stop=True
                )

            # --- per-head input DMAs.  head 0 arrives in fine-grained chunks
            # so the first QK matmuls can start earlier.
            def load_head(hd):
                qT_sb = io.tile([128, S], BF16, tag="qT", name="qT")
                kT_sb = io.tile([128, S], BF16, tag="kT", name="kT")
                v_sb = io.tile([128, NKB, D], FP16, tag="v", name="v")
                dma = nc.default_dma_engine
                if hd == 0:
                    kc, qc, vc = 256, 512, 2
                    for c in range(S // kc):
                        sl = slice(c * kc, (c + 1) * kc)
                        dma.dma_start(out=kT_sb[:, sl], in_=kT_d[hd, :, sl])
                        if c * qc < S:
                            slq = slice(c * qc, (c + 1) * qc)
                            dma.dma_start(out=qT_sb[:, slq], in_=qT_d[hd, :, slq])
                        lo = c * vc
                        dma.dma_start(
                            out=v_sb[:, lo:lo + vc, :], in_=v_d[hd, :, lo:lo + vc, :]
                        )
                else:
                    dma.dma_start(out=qT_sb[:], in_=qT_d[hd])
                    dma.dma_start(out=kT_sb[:], in_=kT_d[hd])
                    dma.dma_start(out=v_sb[:], in_=v_d[hd])
                return qT_sb, kT_sb, v_sb

            heads = {0: load_head(0)}

            p_hist = {}   # chunk idx -> P tile [128, w, 512]
            pend = {}     # (stg, half, level) -> partial fold tile
            acc_cur = {}  # stg -> AV accumulator psum tile
            af_cur = {}   # stg -> fold output tile [128, 1024]

            def emit_qk(i):
                hd, sti, stg, shift, cs, w = chunks[i]
                qT_sb, kT_sb, _ = heads[hd]
                for h in range(cs, cs + w):
                    b, hf = divmod(h, 2)
                    p = (h + shift) % 6
                    nc.tensor.matmul(
                        st6[:, p, :],
                        kT_sb[:, b * KB:(b + 1) * KB],
                        qT_sb[:, sti * QT + hf * 512: sti * QT + (hf + 1) * 512],
                        start=True,
                        stop=True,
                    )

            def emit_act(i):
                hd, sti, stg, shift, cs, w = chunks[i]
                ps0 = (cs + shift) % 6
                p_sb = pexp.tile([128, 3, 512], BF16, tag="p", name="p")
                nc.scalar.activation(
                    p_sb[:, 0:w, :],
                    st6[:, ps0:ps0 + w, :],
                    mybir.ActivationFunctionType.Exp,
                    bias=bias_sb[:, :],
                    scale=1.0,
                )
                p_hist[i] = p_sb

            def emit_av(i):
                hd, sti, stg, shift, cs, w = chunks[i]
                _, _, v_sb = heads[hd]
                p_sb = p_hist[i]
                if cs == 0:
                    acc_cur[stg] = ps.tile(
                        [128, QT], F32, tag="acc", bufs=1, name="acc"
                    )
                acc = acc_cur[stg]
                for j in range(w):
                    b, hf = divmod(cs + j, 2)
                    nc.tensor.matmul(
                        acc[:, hf * 512:(hf + 1) * 512],
                        v_sb[:, b, :],
                        p_sb[:, j, :],
                        start=(b == 0),
                        stop=(b == NKB - 1),
                    )

            def emit_fold(i):
                hd, sti, stg, shift, cs, w = chunks[i]
                p_sb = p_hist.pop(i)
                if cs == 0:
                    af_cur[stg] = foldp.tile(
                        [128, QT], BF16, tag="af", name="af"
                    )
                af = af_cur[stg]
                for j in range(w):
                    _, hf = divmod(cs + j, 2)
                    node = p_sb[:, j, :]
                    level = 0
                    # binary-counter pairwise tree fold (bf16)
                    while (stg, hf, level) in pend:
                        prev = pend.pop((stg, hf, level))
                        level += 1
                        if level == 4:
                            out_ap = af[:, hf * 512:(hf + 1) * 512]
                        else:
                            out_ap = foldp.tile(
                                [128, 512], BF16, tag=f"n{hf}l{level}",
                                name="fn",
                            )[:]
                        nc.vector.tensor_add(out_ap, prev, node)
                        node = out_ap
                    if level < 4:
                        pend[(stg, hf, level)] = node
                if cs + w == NHS:
                    del af_cur[stg]
                    nc.default_dma_engine.dma_start(
                        out=accf_d[hd, sti], in_=af[:]
                    )
                    acc = acc_cur.pop(stg)
                    out_sb = osbp.tile([128, QT], F32, tag="osb", name="osb")
                    nc.vector.tensor_copy(out_sb[:], acc[:])
                    nc.default_dma_engine.dma_start(
                        out=oT_d[hd, :, sti * QT:(sti + 1) * QT], in_=out_sb[:]
                    )

            for i in range(NCH + 2):
                if i < NCH:
                    hd, sti, stg, shift, cs, w = chunks[i]
                    if sti == 0 and cs == 0 and hd + 1 < HPC:
                        heads[hd + 1] = load_head(hd + 1)
                if 1 <= i <= NCH:
                    emit_act(i - 1)
                if i < NCH:
                    emit_qk(i)
                if i >= 2:
                    emit_av(i - 2)
                    emit_fold(i - 2)

    nc.finalize()
    return nc


def _get_nc():
    global _NC_CACHE
    if _NC_CACHE is None:
        _NC_CACHE = _build_nc()
    return _NC_CACHE


def kernel(q, k, v):
    q = np.asarray(q, dtype=np.float32).reshape(B * H, S, D)
    k = np.asarray(k, dtype=np.float32).reshape(B * H, S, D)
    v = np.asarray(v, dtype=np.float32).reshape(B * H, S, D)

    in_maps = []
    for c in range(N_CORES):
        sl = slice(c * HPC, (c + 1) * HPC)
        vh = v[sl].reshape(HPC, NKB, 128, D).transpose(0, 2, 1, 3)
        in_maps.append(
            {
                "qT": np.ascontiguousarray(q[sl].transpose(0, 2, 1)).astype(
                    ml_dtypes.bfloat16
                ),
                "kT": np.ascontiguousarray(k[sl].transpose(0, 2, 1)).astype(
                    ml_dtypes.bfloat16
                ),
                "v": np.ascontiguousarray(vh).astype(np.float16),
            }
        )

    nc = _get_nc()
    trace = bool(int(os.environ.get("KERNEL_TRACE", "0")))
    res = run_bass_kernel_spmd(
        nc, in_maps, core_ids=list(range(N_CORES)), trace=trace
    )
    if trace:
        print(f"HW exec time: {res.exec_time_ns} ns")
        if res.instructions_and_trace:
            print(f"Trace: {res.instructions_and_trace[1]}")

    out = np.empty((B * H, S, D), dtype=np.float32)
    for c in range(N_CORES):
        oT = res.results[c]["outT"]  # [HPC, D, S]
        accf = np.asarray(res.results[c]["acc_fold"]).astype(np.float32)
        # final 128-partition fold of the device-side pairwise-folded P
        l = accf.sum(axis=2).reshape(HPC, S)  # [HPC, NQT, QT] -> [HPC, S]
        out[c * HPC:(c + 1) * HPC] = oT.transpose(0, 2, 1) / l[:, :, None]
    return out.reshape(B, H, S, D)
